# revision 16
# baseline (speedup 1.0000x reference)
"""GQA multi-head attention (B=4,T=2048,E=1024, 8 q-heads / 4 kv-heads, RoPE,
causal) on 8 TRN2 NeuronCores.

Sharding: data-parallel over batch (4) x tensor-parallel over head groups (2).
Core c = 2*b + g handles batch b with q-heads [4g..4g+4) / kv-heads [2g..2g+2).
Each core computes a partial o_proj output; the host sums the two partials per
batch (the all-reduce of the column-sharded o_proj).

Device pipeline per core:
  1. fused qkv matmul in natural [t, f] layout (x-chunk stationary, streamed),
     RoPE on DVE (pairs along free dim), DMA-xbar-transpose q/k heads into
     [d, t] layout.
  2. per q head: scores computed transposed ([s,t] = k-stationary x qT), exp
     on ACT straight out of PSUM (scale fused) into bf16 attnT chunks -- which
     are directly the PV stationary operand.  Causal at 128 granularity.
     Head h scores are interleaved with head h-1 PV so PE never waits on ACT.
  3. PV with a ones-column appended to v: psum [t, d+1] accumulates out and
     the softmax denominator; normalize with DVE reciprocal+scale; DMA-xbar
     transpose attnout into [f, t] for o_proj.
  4. o_proj partials [t, 1024] -> DRAM fp32, interleaved with last head's PV.
"""

import numpy as np
import ml_dtypes
from contextlib import ExitStack

import concourse.bass as bass
import concourse.tile as tile
from concourse import bacc, mybir
from concourse.bass import ts, ds
from concourse.bass_utils import run_bass_kernel_spmd
from concourse.masks import make_identity

BF16 = mybir.dt.bfloat16
F32 = mybir.dt.float32

B, T, E = 4, 2048, 1024
HQ, HKV, D = 8, 4, 128
G = HQ // HKV          # 2 q heads per kv head
NGRP = 2               # tensor-parallel head groups
QH = HQ // NGRP        # 4 local q heads
KH = HKV // NGRP       # 2 local kv heads
NH = QH + KH           # 6 rope'd heads
FQ = QH * D            # 512 local q features
FK = KH * D            # 256 local kv features
FA = FQ + 2 * FK       # 1024 fused qkv features
EO = E // 128          # 8 contraction chunks
NTB = T // 128         # 16 t/s blocks of 128
DH = D // 2            # 64 rope half
SCALE = 1.0 / float(np.sqrt(D))
NCORES = 8
EXP = mybir.ActivationFunctionType.Exp


def _body(tc, ctx, io):
    nc = tc.nc
    xT, wqkvT, woT, cosN, sinN, maskT, out = io

    const = ctx.enter_context(tc.tile_pool(name="const", bufs=1))
    pers = ctx.enter_context(tc.tile_pool(name="pers", bufs=1))
    tmp = ctx.enter_context(tc.tile_pool(name="tmp", bufs=3))
    xcp = ctx.enter_context(tc.tile_pool(name="xcp", bufs=2))
    attnp = ctx.enter_context(tc.tile_pool(name="attnp", bufs=8))
    outp = ctx.enter_context(tc.tile_pool(name="outp", bufs=2))
    ps_sc = ctx.enter_context(tc.tile_pool(name="ps_sc", bufs=3, space="PSUM"))
    ps_b1 = ctx.enter_context(tc.tile_pool(name="ps_b1", bufs=2, space="PSUM"))

    # ---- persistent loads ----
    wqkv_sb = pers.tile([128, EO, FA], BF16)
    wo_sb = pers.tile([128, FQ // 128, E], BF16)
    cos_sb = const.tile([128, NTB, DH], BF16)
    sin_sb = const.tile([128, NTB, DH], BF16)
    mask_sb = const.tile([128, 128], BF16)
    ident = const.tile([128, 128], BF16)
    make_identity(nc, ident[:])

    nc.sync.dma_start(cos_sb[:], cosN.rearrange("p (tb i) -> p tb i", i=DH))
    nc.sync.dma_start(sin_sb[:], sinN.rearrange("p (tb i) -> p tb i", i=DH))
    nc.sync.dma_start(mask_sb[:], maskT[:])

    qkT_sb = pers.tile([128, NH, T], BF16)
    # v in natural [s, d] layout, with a ones column appended per kv head
    vaug_sb = pers.tile([128, NTB, KH, D + 1], BF16)
    nc.vector.memset(vaug_sb[:, :, :, D : D + 1], 1.0)
    attnoutT_sb = pers.tile([128, QH, T], BF16)

    def emit_phase1():
        pass
    # ---- phase 1: fused qkv projection (natural layout) + rope + transpose
    # Reversed block order so attention chunks (which need qT[t>=sc*128])
    # become ready early, letting heads 0/1 exp overlap the projections.
    first_tq = True
    for tq in range(T // 512):
        xc = xcp.tile([128, EO, 512], BF16, tag="xc", name="xc")
        for eo in range(EO):
            nc.sync.dma_start(xc[:, eo, :], xT[ds(eo * 128, 128), ts(tq, 512)])
            if first_tq:
                nc.sync.dma_start(wqkv_sb[:, eo, :], wqkvT[ds(eo * 128, 128), :])
        first_tq = False
        for tbl in range(4):
            tb = tq * 4 + tbl
            pq = ps_sc.tile([128, FA], F32, tag="ps_sc", name="pq")
            for eo in range(EO):
                for half in range(2):
                    nc.tensor.matmul(
                        pq[:, ds(half * 512, 512)],
                        xc[:, eo, ts(tbl, 128)],
                        wqkv_sb[:, eo, ds(half * 512, 512)],
                        start=(eo == 0), stop=(eo == EO - 1))
            raw = tmp.tile([128, FA], BF16, tag="raw", name="raw", bufs=2)
            nc.scalar.copy(raw[:], pq[:])
            # rope on q + k heads, batched across heads via 3d strided APs
            raw3 = raw.rearrange("p (h d) -> p h d", d=D)
            rot = tmp.tile([128, NH, D], BF16, tag="rot", name="rot", bufs=2)
            c6 = cos_sb[:, tb, None, :].to_broadcast((128, NH, DH))
            s6 = sin_sb[:, tb, None, :].to_broadcast((128, NH, DH))
            x1 = raw3[:, 0:NH, 0:DH]
            x2 = raw3[:, 0:NH, DH:D]
            t1 = tmp.tile([128, NH, DH], BF16, tag="t1", name="t1", bufs=2)
            o1 = rot[:, :, 0:DH]
            o2 = rot[:, :, DH:D]
            nc.vector.tensor_mul(o1, x1, c6)
            nc.vector.tensor_mul(t1[:], x2, s6)
            nc.vector.tensor_sub(o1, o1, t1[:])
            nc.vector.tensor_mul(o2, x1, s6)
            nc.vector.tensor_mul(t1[:], x2, c6)
            nc.vector.tensor_add(o2, o2, t1[:])
            # transpose rope'd heads into [d, t] layout via PE, one batched copy
            ptq = ps_b1.tile([128, NH, 128], BF16, tag="ps_b1", name="ptq")
            for hh in range(NH):
                nc.tensor.transpose(ptq[:, hh, :], rot[:, hh, :], ident[:])
            nc.vector.tensor_copy(qkT_sb[:, :, ts(tb, 128)], ptq[:])
            # v slices (already natural)
            nc.vector.tensor_copy(
                vaug_sb[:, tb, :, 0:D],
                raw3[:, NH : NH + KH, :])
            # heads 0/1 attention chunks for sc=tb are now data-complete
            scores_chunk(0, tb, attn_chunks[0])
            scores_chunk(1, tb, attn_chunks[1])

    # ---- attention helpers (used by phase 1 interleave and phase 2) ----
    def scores_chunk(h, sc, alist):
        kv = h // G
        tv0 = (sc // 4) * 512          # tile left edge (512-aligned)
        w = T - tv0
        vs = sc * 128 - tv0            # valid start within tile (causal)
        at = attnp.tile([128, w], BF16, tag=f"attn{w}", name=f"at{h}_{sc}")
        for t0 in range(tv0, T, 1024):
            span = min(1024, T - t0)
            lo = vs if t0 == tv0 else 0
            pss = ps_sc.tile([128, 1024], F32, tag="ps_sc", name="pss")
            u = lo
            while u < span:
                uw = min(512 - (u % 512), span - u)
                nc.tensor.matmul(
                    pss[:, u:u + uw],
                    kT_sb[kv][:, ts(sc, 128)],
                    qT_sb[h][:, ds(t0 + u, uw)],
                    start=True, stop=True)
                u += uw
            nc.scalar.activation(
                at[:, ds(t0 - tv0 + lo, span - lo)], pss[:, lo:span],
                EXP, scale=SCALE)
        nc.vector.tensor_mul(at[:, ds(vs, 128)], at[:, ds(vs, 128)], mask_sb[:])
        alist[sc] = at

    def pv_block(h, tb, alist):
        kv = h // G
        po = ps_b1.tile([128, D + 1], F32, tag="ps_b1", name="po")
        for sc in range(tb + 1):
            tv0 = (sc // 4) * 512
            nc.tensor.matmul(
                po[:], alist[sc][:, ds(tb * 128 - tv0, 128)],
                vaug_sb[:, sc, kv, :],
                start=(sc == 0), stop=(sc == tb))
        rs = tmp.tile([128, 1], F32, tag="rs", name="rs")
        nc.vector.reciprocal(rs[:], po[:, D : D + 1])
        ao = tmp.tile([128, D], BF16, tag="ao", name="ao")
        nc.vector.tensor_scalar_mul(ao[:], po[:, 0:D], rs[:])
        pt2 = ps_b1.tile([128, 128], BF16, tag="ps_b1", name="pt2")
        nc.tensor.transpose(pt2[:], ao[:], ident[:])
        nc.vector.tensor_copy(attnoutT_sb[:, h, ts(tb, 128)], pt2[:])

    def oproj_block(tb, fos, accum):
        ob = outp.tile([128, E], F32, tag="ob", name="ob")
        psh = [ps_sc.tile([128, 512], F32, tag="ps_sc", name=f"pso{i}")
               for i in range(2)]
        for j, fo in enumerate(fos):
            for half in range(2):
                nc.tensor.matmul(
                    psh[half][:],
                    attnoutT_sb[:, fo, ts(tb, 128)],
                    wo_sb[:, fo, ds(half * 512, 512)],
                    start=(j == 0), stop=(j == len(fos) - 1))
        nc.vector.tensor_copy(ob[:, 0:512], psh[0][:])
        if accum:
            nc.scalar.copy(ob[:, 512:E], psh[1][:])
            nc.gpsimd.dma_start(out[ds(tb * 128, 128), :], ob[:],
                                accum_op=mybir.AluOpType.add)
        else:
            nc.vector.tensor_copy(ob[:, 512:E], psh[1][:])
            nc.sync.dma_start(out[ds(tb * 128, 128), :], ob[:])

    attn_chunks = [dict() for _ in range(QH)]
    emit_phase1()
    for fo in range(FQ // 128):
        nc.sync.dma_start(wo_sb[:, fo, :], woT[ds(fo * 128, 128), :])
    for i in range(NTB):
        pv_block(0, i, attn_chunks[0])
    for i in range(NTB):
        scores_chunk(2, i, attn_chunks[2])
    for i in range(NTB):
        pv_block(1, i, attn_chunks[1])
    for i in range(NTB):
        scores_chunk(3, i, attn_chunks[3])
    for i in range(NTB):
        pv_block(2, i, attn_chunks[2])
    for i in range(NTB):
        pv_block(3, i, attn_chunks[3])
        oproj_block(i)


def build():
    nc = bacc.Bacc("TRN2", target_bir_lowering=False, debug=False,
                   enable_asserts=False)
    xT = nc.dram_tensor("xT", [E, T], BF16, kind="ExternalInput").ap()
    wqkvT = nc.dram_tensor("wqkvT", [E, FA], BF16, kind="ExternalInput").ap()
    woT = nc.dram_tensor("woT", [FQ, E], BF16, kind="ExternalInput").ap()
    cosN = nc.dram_tensor("cosN", [128, NTB * DH], BF16, kind="ExternalInput").ap()
    sinN = nc.dram_tensor("sinN", [128, NTB * DH], BF16, kind="ExternalInput").ap()
    maskT = nc.dram_tensor("maskT", [128, 128], BF16, kind="ExternalInput").ap()
    out = nc.dram_tensor("out", [T, E], F32, kind="ExternalOutput").ap()
    io = (xT, wqkvT, woT, cosN, sinN, maskT, out)
    with tile.TileContext(nc) as tc, ExitStack() as ctx:
        _body(tc, ctx, io)
    nc.compile()
    return nc


_NC_CACHE = None


def _get_nc():
    global _NC_CACHE
    if _NC_CACHE is None:
        _NC_CACHE = build()
    return _NC_CACHE


def _bf16(a):
    return np.ascontiguousarray(a).astype(ml_dtypes.bfloat16)


def make_in_maps(x, Wq, Wk, Wv, Wo):
    inv_freq = (1.0 / (10000.0 ** (2.0 * np.arange(DH, dtype=np.float32) / D)))
    theta = np.arange(T, dtype=np.float32)[:, None] * inv_freq[None, :]
    # pre-tiled [128, NTB*DH]: row p holds cos(theta[tb*128+p, :]) for each tb
    cosN = _bf16(np.cos(theta).reshape(NTB, 128, DH).transpose(1, 0, 2).reshape(128, NTB * DH))
    sinN = _bf16(np.sin(theta).reshape(NTB, 128, DH).transpose(1, 0, 2).reshape(128, NTB * DH))
    ls = np.arange(128)
    maskT = _bf16((ls[:, None] <= ls[None, :]).astype(np.float32))  # s<=t valid
    in_maps = []
    for c in range(NCORES):
        b, g = c // NGRP, c % NGRP
        wq = Wq[g * FQ:(g + 1) * FQ, :]      # [512, 1024]
        wk = Wk[g * FK:(g + 1) * FK, :]      # [256, 1024]
        wv = Wv[g * FK:(g + 1) * FK, :]      # [256, 1024]
        wqkv = np.concatenate([wq, wk, wv], axis=0)   # [1024, 1024]
        in_maps.append({
            "xT": _bf16(x[b].T),
            "wqkvT": _bf16(wqkv.T),
            "woT": _bf16(Wo[:, g * FQ:(g + 1) * FQ].T),
            "cosN": cosN, "sinN": sinN, "maskT": maskT,
        })
    return in_maps


def kernel(x, Wq, Wk, Wv, Wo, _trace=False):
    nc = _get_nc()
    in_maps = make_in_maps(np.asarray(x, dtype=np.float32),
                           np.asarray(Wq, dtype=np.float32),
                           np.asarray(Wk, dtype=np.float32),
                           np.asarray(Wv, dtype=np.float32),
                           np.asarray(Wo, dtype=np.float32))
    res = run_bass_kernel_spmd(nc, in_maps, core_ids=list(range(NCORES)),
                               trace=_trace)
    outs = [r["out"].astype(np.float32) for r in res.results]
    full = np.stack([outs[2 * b] + outs[2 * b + 1] for b in range(B)], axis=0)
    if _trace:
        kernel.last_exec_time_ns = res.exec_time_ns
        kernel.last_results = res
    return full


# revision 17
# speedup vs baseline: 1.2465x; 1.2465x over previous
"""GQA multi-head attention (B=4,T=2048,E=1024, 8 q-heads / 4 kv-heads, RoPE,
causal) on 8 TRN2 NeuronCores.

Sharding: data-parallel over batch (4) x tensor-parallel over head groups (2).
Core c = 2*b + g handles batch b with q-heads [4g..4g+4) / kv-heads [2g..2g+2).
Each core computes a partial o_proj output; the host sums the two partials per
batch (the all-reduce of the column-sharded o_proj).

Device pipeline per core:
  1. fused qkv matmul in natural [t, f] layout (x-chunk stationary, streamed),
     RoPE on DVE (pairs along free dim), DMA-xbar-transpose q/k heads into
     [d, t] layout.
  2. per q head: scores computed transposed ([s,t] = k-stationary x qT), exp
     on ACT straight out of PSUM (scale fused) into bf16 attnT chunks -- which
     are directly the PV stationary operand.  Causal at 128 granularity.
     Head h scores are interleaved with head h-1 PV so PE never waits on ACT.
  3. PV with a ones-column appended to v: psum [t, d+1] accumulates out and
     the softmax denominator; normalize with DVE reciprocal+scale; DMA-xbar
     transpose attnout into [f, t] for o_proj.
  4. o_proj partials [t, 1024] -> DRAM fp32, interleaved with last head's PV.
"""

import numpy as np
import ml_dtypes
from contextlib import ExitStack

import concourse.bass as bass
import concourse.tile as tile
from concourse import bacc, mybir
from concourse.bass import ts, ds
from concourse.bass_utils import run_bass_kernel_spmd
from concourse.masks import make_identity

BF16 = mybir.dt.bfloat16
F32 = mybir.dt.float32

B, T, E = 4, 2048, 1024
HQ, HKV, D = 8, 4, 128
G = HQ // HKV          # 2 q heads per kv head
NGRP = 2               # tensor-parallel head groups
QH = HQ // NGRP        # 4 local q heads
KH = HKV // NGRP       # 2 local kv heads
NH = QH + KH           # 6 rope'd heads
FQ = QH * D            # 512 local q features
FK = KH * D            # 256 local kv features
FA = FQ + 2 * FK       # 1024 fused qkv features
EO = E // 128          # 8 contraction chunks
NTB = T // 128         # 16 t/s blocks of 128
DH = D // 2            # 64 rope half
SCALE = 1.0 / float(np.sqrt(D))
NCORES = 8
EXP = mybir.ActivationFunctionType.Exp


def _body(tc, ctx, io):
    nc = tc.nc
    xT, wqkvT, woT, cosN, sinN, maskT, out = io

    const = ctx.enter_context(tc.tile_pool(name="const", bufs=1))
    pers = ctx.enter_context(tc.tile_pool(name="pers", bufs=1))
    tmp = ctx.enter_context(tc.tile_pool(name="tmp", bufs=3))
    xcp = ctx.enter_context(tc.tile_pool(name="xcp", bufs=2))
    attnp = ctx.enter_context(tc.tile_pool(name="attnp", bufs=8))
    outp = ctx.enter_context(tc.tile_pool(name="outp", bufs=2))
    ps_sc = ctx.enter_context(tc.tile_pool(name="ps_sc", bufs=3, space="PSUM"))
    ps_b1 = ctx.enter_context(tc.tile_pool(name="ps_b1", bufs=2, space="PSUM"))

    # ---- persistent loads ----
    wqkv_sb = pers.tile([128, EO, FA], BF16)
    wo_sb = pers.tile([128, FQ // 128, E], BF16)
    cos_sb = const.tile([128, NTB, DH], BF16)
    sin_sb = const.tile([128, NTB, DH], BF16)
    mask_sb = const.tile([128, 128], BF16)
    ident = const.tile([128, 128], BF16)
    make_identity(nc, ident[:])

    nc.sync.dma_start(cos_sb[:], cosN.rearrange("p (tb i) -> p tb i", i=DH))
    nc.sync.dma_start(sin_sb[:], sinN.rearrange("p (tb i) -> p tb i", i=DH))
    nc.sync.dma_start(mask_sb[:], maskT[:])

    qkT_sb = pers.tile([128, NH, T], BF16)
    # v in natural [s, d] layout, with a ones column appended per kv head
    vaug_sb = pers.tile([128, NTB, KH, D + 1], BF16)
    nc.vector.memset(vaug_sb[:, :, :, D : D + 1], 1.0)
    attnoutT_sb = pers.tile([128, QH, T], BF16)

    def emit_phase1():
        pass
    # ---- phase 1: fused qkv projection (natural layout) + rope + transpose
    # Reversed block order so attention chunks (which need qT[t>=sc*128])
    # become ready early, letting heads 0/1 exp overlap the projections.
    first_tq = True
    for tq in range(T // 512):
        xc = xcp.tile([128, EO, 512], BF16, tag="xc", name="xc")
        for eo in range(EO):
            nc.sync.dma_start(xc[:, eo, :], xT[ds(eo * 128, 128), ts(tq, 512)])
            if first_tq:
                nc.sync.dma_start(wqkv_sb[:, eo, :], wqkvT[ds(eo * 128, 128), :])
        first_tq = False
        for tbl in range(4):
            tb = tq * 4 + tbl
            pq = ps_sc.tile([128, FA], F32, tag="ps_sc", name="pq")
            for eo in range(EO):
                for half in range(2):
                    nc.tensor.matmul(
                        pq[:, ds(half * 512, 512)],
                        xc[:, eo, ts(tbl, 128)],
                        wqkv_sb[:, eo, ds(half * 512, 512)],
                        start=(eo == 0), stop=(eo == EO - 1))
            raw = tmp.tile([128, FA], BF16, tag="raw", name="raw", bufs=2)
            nc.scalar.copy(raw[:], pq[:])
            # rope on q + k heads, batched across heads via 3d strided APs
            raw3 = raw.rearrange("p (h d) -> p h d", d=D)
            rot = tmp.tile([128, NH, D], BF16, tag="rot", name="rot", bufs=2)
            c6 = cos_sb[:, tb, None, :].to_broadcast((128, NH, DH))
            s6 = sin_sb[:, tb, None, :].to_broadcast((128, NH, DH))
            x1 = raw3[:, 0:NH, 0:DH]
            x2 = raw3[:, 0:NH, DH:D]
            t1 = tmp.tile([128, NH, DH], BF16, tag="t1", name="t1", bufs=2)
            o1 = rot[:, :, 0:DH]
            o2 = rot[:, :, DH:D]
            nc.vector.tensor_mul(o1, x1, c6)
            nc.vector.tensor_mul(t1[:], x2, s6)
            nc.vector.tensor_sub(o1, o1, t1[:])
            nc.vector.tensor_mul(o2, x1, s6)
            nc.vector.tensor_mul(t1[:], x2, c6)
            nc.vector.tensor_add(o2, o2, t1[:])
            # transpose rope'd heads into [d, t] layout via PE, one batched copy
            ptq = ps_b1.tile([128, NH, 128], BF16, tag="ps_b1", name="ptq")
            for hh in range(NH):
                nc.tensor.transpose(ptq[:, hh, :], rot[:, hh, :], ident[:])
            nc.vector.tensor_copy(qkT_sb[:, :, ts(tb, 128)], ptq[:])
            # v slices (already natural)
            nc.vector.tensor_copy(
                vaug_sb[:, tb, :, 0:D],
                raw3[:, NH : NH + KH, :])
            # heads 0/1 attention chunks for sc=tb are now data-complete
            scores_chunk(0, tb, attn_chunks[0])
            scores_chunk(1, tb, attn_chunks[1])

    # ---- attention helpers (used by phase 1 interleave and phase 2) ----
    def scores_chunk(h, sc, alist):
        kv = h // G
        tv0 = (sc // 4) * 512          # tile left edge (512-aligned)
        w = T - tv0
        vs = sc * 128 - tv0            # valid start within tile (causal)
        at = attnp.tile([128, w], BF16, tag=f"attn{w}", name=f"at{h}_{sc}")
        for t0 in range(tv0, T, 1024):
            span = min(1024, T - t0)
            lo = vs if t0 == tv0 else 0
            pss = ps_sc.tile([128, 1024], F32, tag="ps_sc", name="pss")
            u = lo
            while u < span:
                uw = min(512 - (u % 512), span - u)
                nc.tensor.matmul(
                    pss[:, u:u + uw],
                    kT_sb[kv][:, ts(sc, 128)],
                    qT_sb[h][:, ds(t0 + u, uw)],
                    start=True, stop=True)
                u += uw
            nc.scalar.activation(
                at[:, ds(t0 - tv0 + lo, span - lo)], pss[:, lo:span],
                EXP, scale=SCALE)
        nc.vector.tensor_mul(at[:, ds(vs, 128)], at[:, ds(vs, 128)], mask_sb[:])
        alist[sc] = at

    def pv_block(h, tb, alist):
        kv = h // G
        po = ps_b1.tile([128, D + 1], F32, tag="ps_b1", name="po")
        for sc in range(tb + 1):
            tv0 = (sc // 4) * 512
            nc.tensor.matmul(
                po[:], alist[sc][:, ds(tb * 128 - tv0, 128)],
                vaug_sb[:, sc, kv, :],
                start=(sc == 0), stop=(sc == tb))
        rs = tmp.tile([128, 1], F32, tag="rs", name="rs")
        nc.vector.reciprocal(rs[:], po[:, D : D + 1])
        ao = tmp.tile([128, D], BF16, tag="ao", name="ao")
        nc.vector.tensor_scalar_mul(ao[:], po[:, 0:D], rs[:])
        pt2 = ps_b1.tile([128, 128], BF16, tag="ps_b1", name="pt2")
        nc.tensor.transpose(pt2[:], ao[:], ident[:])
        nc.vector.tensor_copy(attnoutT_sb[:, h, ts(tb, 128)], pt2[:])

    def oproj_block(tb):
        ob = outp.tile([128, E], F32, tag="ob", name="ob")
        psh = [ps_sc.tile([128, 512], F32, tag="ps_sc", name=f"pso{i}")
               for i in range(2)]
        for fo in range(QH):
            for half in range(2):
                nc.tensor.matmul(
                    psh[half][:],
                    attnoutT_sb[:, fo, ts(tb, 128)],
                    wo_sb[:, fo, ds(half * 512, 512)],
                    start=(fo == 0), stop=(fo == QH - 1))
        nc.vector.tensor_copy(ob[:, 0:512], psh[0][:])
        nc.scalar.copy(ob[:, 512:E], psh[1][:])
        nc.sync.dma_start(out[ds(tb * 128, 128), :], ob[:])

    attn_chunks = [dict() for _ in range(QH)]
    emit_phase1()
    for fo in range(FQ // 128):
        nc.sync.dma_start(wo_sb[:, fo, :], woT[ds(fo * 128, 128), :])
    for i in range(NTB):
        pv_block(0, i, attn_chunks[0])
    for i in range(NTB):
        scores_chunk(2, i, attn_chunks[2])
    for i in range(NTB):
        pv_block(1, i, attn_chunks[1])
    for i in range(NTB):
        scores_chunk(3, i, attn_chunks[3])
    for i in range(NTB):
        pv_block(2, i, attn_chunks[2])
    for i in range(NTB):
        pv_block(3, i, attn_chunks[3])
        oproj_block(i)


def build():
    nc = bacc.Bacc("TRN2", target_bir_lowering=False, debug=False,
                   enable_asserts=False)
    xT = nc.dram_tensor("xT", [E, T], BF16, kind="ExternalInput").ap()
    wqkvT = nc.dram_tensor("wqkvT", [E, FA], BF16, kind="ExternalInput").ap()
    woT = nc.dram_tensor("woT", [FQ, E], BF16, kind="ExternalInput").ap()
    cosN = nc.dram_tensor("cosN", [128, NTB * DH], BF16, kind="ExternalInput").ap()
    sinN = nc.dram_tensor("sinN", [128, NTB * DH], BF16, kind="ExternalInput").ap()
    maskT = nc.dram_tensor("maskT", [128, 128], BF16, kind="ExternalInput").ap()
    out = nc.dram_tensor("out", [T, E], F32, kind="ExternalOutput").ap()
    io = (xT, wqkvT, woT, cosN, sinN, maskT, out)
    with tile.TileContext(nc) as tc, ExitStack() as ctx:
        _body(tc, ctx, io)
    nc.compile()
    return nc


_NC_CACHE = None


def _get_nc():
    global _NC_CACHE
    if _NC_CACHE is None:
        _NC_CACHE = build()
    return _NC_CACHE


def _bf16(a):
    return np.ascontiguousarray(a).astype(ml_dtypes.bfloat16)


def make_in_maps(x, Wq, Wk, Wv, Wo):
    inv_freq = (1.0 / (10000.0 ** (2.0 * np.arange(DH, dtype=np.float32) / D)))
    theta = np.arange(T, dtype=np.float32)[:, None] * inv_freq[None, :]
    # pre-tiled [128, NTB*DH]: row p holds cos(theta[tb*128+p, :]) for each tb
    cosN = _bf16(np.cos(theta).reshape(NTB, 128, DH).transpose(1, 0, 2).reshape(128, NTB * DH))
    sinN = _bf16(np.sin(theta).reshape(NTB, 128, DH).transpose(1, 0, 2).reshape(128, NTB * DH))
    ls = np.arange(128)
    maskT = _bf16((ls[:, None] <= ls[None, :]).astype(np.float32))  # s<=t valid
    in_maps = []
    for c in range(NCORES):
        b, g = c // NGRP, c % NGRP
        wq = Wq[g * FQ:(g + 1) * FQ, :]      # [512, 1024]
        wk = Wk[g * FK:(g + 1) * FK, :]      # [256, 1024]
        wv = Wv[g * FK:(g + 1) * FK, :]      # [256, 1024]
        wqkv = np.concatenate([wq, wk, wv], axis=0)   # [1024, 1024]
        in_maps.append({
            "xT": _bf16(x[b].T),
            "wqkvT": _bf16(wqkv.T),
            "woT": _bf16(Wo[:, g * FQ:(g + 1) * FQ].T),
            "cosN": cosN, "sinN": sinN, "maskT": maskT,
        })
    return in_maps


def kernel(x, Wq, Wk, Wv, Wo, _trace=False):
    nc = _get_nc()
    in_maps = make_in_maps(np.asarray(x, dtype=np.float32),
                           np.asarray(Wq, dtype=np.float32),
                           np.asarray(Wk, dtype=np.float32),
                           np.asarray(Wv, dtype=np.float32),
                           np.asarray(Wo, dtype=np.float32))
    res = run_bass_kernel_spmd(nc, in_maps, core_ids=list(range(NCORES)),
                               trace=_trace)
    outs = [r["out"].astype(np.float32) for r in res.results]
    full = np.stack([outs[2 * b] + outs[2 * b + 1] for b in range(B)], axis=0)
    if _trace:
        kernel.last_exec_time_ns = res.exec_time_ns
        kernel.last_results = res
    return full
